# revision 8
# baseline (speedup 1.0000x reference)
"""Trainium2 Bass kernel for nn_Cross_Attention_Block_3624952397825.

Mathematical structure exploited: the reference takes ``out[:, -1, :]`` --
the attention output of the LAST query token. That token comes from the
zero row appended by ``jnp.pad`` AFTER the conv stack, so its query vector
is exactly zero, its attention scores are exactly zero, and softmax over
exact zeros is exactly uniform (1/4096).  Hence

    bins[b]  = mean_k V[b, k, :]          = (mean_k lidar[b, k, :]) @ wv
    out[b]   = MLP3(leaky_relu chain)(bins[b])

The conv block, Q/K projections, and softmax are structurally dead code
for ANY input values.  The kernel therefore reduces lidar over its 4096
points on-device, applies wv (pre-scaled by 1/4096 on the host) and the
3-layer MLP, all on 8 NeuronCores data-parallel over the batch (2
batches per core).

Layout notes:
  * lidar shard [2, 4096, 256] is streamed as 8 x [128, 2048] tiles
    (1 MiB DMAs, 8 KiB contiguous per partition) and accumulated with
    VectorE adds; free-dim folding brings per-lane partials to
    [128, 256]; a ones-column matmul folds the 128 partitions.
  * Activations flow through the MLP transposed ([features, batch]) so
    per-channel biases are per-partition ScalarE activation biases.
  * All MLP weights + biases + constants ship in one packed [128, 3463]
    tensor -> a single weight DMA.
"""

import numpy as np

B, NPTS, CH, DM = 16, 4096, 256, 1024
N_CORES = 8
BL = B // N_CORES            # batches per core
P = 128
TILE_F = 2048                # free dim of each lidar tile (8 pts x 256 ch)
N_TILES = NPTS * CH // (P * TILE_F)   # 4 tiles per batch

# wpack free-dim layout
OFF_WVS = 0                  # 2 k-chunks x 1024
OFF_WO1 = 2048               # 8 k-chunks x 128
OFF_WO2 = 3072               # 128
OFF_WO3 = 3200               # 256
OFF_B = 3456                 # b1, b2, b3[:128], b3[128:]
OFF_ONE = 3460               # column of ones
OFF_I2 = 3461                # 2x2 identity in rows 0..1
WPACK_F = 3463

_CACHE = {}


def _build_program():
    import concourse.bacc as bacc
    import concourse.mybir as mybir
    from concourse.tile import TileContext

    f32 = mybir.dt.float32
    Alu = mybir.AluOpType
    Act = mybir.ActivationFunctionType

    nc = bacc.Bacc("TRN2")
    lidar = nc.dram_tensor("lidar", [BL, NPTS, CH], f32, kind="ExternalInput")
    wpack = nc.dram_tensor("wpack", [P, WPACK_F], f32, kind="ExternalInput")
    outT = nc.dram_tensor("outT", [CH, BL], f32, kind="ExternalOutput")

    # [BL, 4096, 256] -> [(b t), 128, 2048]; per-partition rows are 8 KiB
    # contiguous in DRAM.
    lv = lidar[:, :, :].rearrange("b (t p q) c -> (b t) p (q c)", p=P, q=8)

    with TileContext(nc) as tc:
        with (
            tc.tile_pool(name="w", bufs=1) as wpool,
            tc.tile_pool(name="io", bufs=4) as iopool,
            tc.tile_pool(name="acc", bufs=1) as accpool,
            tc.tile_pool(name="small", bufs=1) as spool,
            tc.tile_pool(name="ps", bufs=2, space="PSUM") as pspool,
            tc.tile_pool(name="mm", bufs=3, space="PSUM") as mmpool,
        ):
            wp = wpool.tile([P, WPACK_F], f32, tag="wp")
            nc.sync.dma_start(out=wp[:, :], in_=wpack[:, :])

            # transposed means land here: mt_ps[k][:, b] = sums[b, k*128:(k+1)*128]
            mt_ps = [mmpool.tile([P, BL], f32, tag=f"mtp{k}", name=f"mtp{k}", bufs=1)
                     for k in range(2)]
            for b in range(BL):
                acc = accpool.tile([P, TILE_F], f32, tag=f"acc{b}")
                first = None
                for t in range(N_TILES):
                    tin = iopool.tile([P, TILE_F], f32, tag="tin")
                    nc.sync.dma_start(out=tin[:, :], in_=lv[b * N_TILES + t, :, :])
                    if t == 0:
                        first = tin
                    elif t == 1:
                        nc.vector.tensor_add(out=acc[:, :], in0=first[:, :], in1=tin[:, :])
                    else:
                        nc.vector.tensor_add(out=acc[:, :], in0=acc[:, :], in1=tin[:, :])
                # fold 2048 -> 256 (free layout is 8 points x 256 channels)
                nc.vector.tensor_add(out=acc[:, 0:1024], in0=acc[:, 0:1024], in1=acc[:, 1024:2048])
                nc.vector.tensor_add(out=acc[:, 0:512], in0=acc[:, 0:512], in1=acc[:, 512:1024])
                af = spool.tile([P, CH], f32, tag=f"af{b}")
                nc.vector.tensor_add(out=af[:, :], in0=acc[:, 0:256], in1=acc[:, 256:512])
                # fold 128 partitions with a ones-column matmul -> [1, 256]
                s_ps = pspool.tile([1, CH], f32, tag="sps")
                nc.tensor.matmul(s_ps[:, :], lhsT=wp[:, OFF_ONE:OFF_ONE + 1],
                                 rhs=af[:, :], start=True, stop=True)
                s_sb = spool.tile([1, CH], f32, tag=f"ssb{b}")
                nc.scalar.copy(out=s_sb[:, :], in_=s_ps[:, :])
                # transpose row [1, 256] into PSUM columns via K=1 matmuls
                for k in range(2):
                    nc.tensor.matmul(mt_ps[k][:, b:b + 1],
                                     lhsT=s_sb[0:1, k * P:(k + 1) * P],
                                     rhs=wp[0:1, OFF_ONE:OFF_ONE + 1],
                                     start=True, stop=True)

            mt = []
            for k in range(2):
                mt_sb = spool.tile([P, BL], f32, tag=f"mt{k}")
                nc.scalar.copy(out=mt_sb[:, :], in_=mt_ps[k][:, :])
                mt.append(mt_sb)

            # v = sums @ (wv/4096): [1024, BL] transposed, 8 output chunks
            vS = spool.tile([P, 8 * BL], f32, tag="vS")
            for o in range(8):
                vps = mmpool.tile([P, BL], f32, tag="mm")
                for k in range(2):
                    nc.tensor.matmul(vps[:, :],
                                     lhsT=wp[:, OFF_WVS + k * 1024 + o * P: OFF_WVS + k * 1024 + (o + 1) * P],
                                     rhs=mt[k][:, :], start=(k == 0), stop=(k == 1))
                nc.scalar.copy(out=vS[:, o * BL:(o + 1) * BL], in_=vps[:, :])

            def leaky(zp, bias_col, tag):
                z = spool.tile([P, BL], f32, tag=f"z{tag}")
                nc.scalar.activation(z[:, :], zp[:, :], Act.Identity,
                                     bias=wp[:, bias_col:bias_col + 1], scale=1.0)
                h = spool.tile([P, BL], f32, tag=f"h{tag}")
                nc.vector.scalar_tensor_tensor(out=h[:, :], in0=z[:, :], scalar=0.01,
                                               in1=z[:, :], op0=Alu.mult, op1=Alu.max)
                return h

            h1p = mmpool.tile([P, BL], f32, tag="mm")
            for k in range(8):
                nc.tensor.matmul(h1p[:, :], lhsT=wp[:, OFF_WO1 + k * P: OFF_WO1 + (k + 1) * P],
                                 rhs=vS[:, k * BL:(k + 1) * BL], start=(k == 0), stop=(k == 7))
            h1 = leaky(h1p, OFF_B + 0, "1")

            h2p = mmpool.tile([P, BL], f32, tag="mm")
            nc.tensor.matmul(h2p[:, :], lhsT=wp[:, OFF_WO2:OFF_WO2 + P],
                             rhs=h1[:, :], start=True, stop=True)
            h2 = leaky(h2p, OFF_B + 1, "2")

            for o in range(2):
                ops = mmpool.tile([P, BL], f32, tag="mm")
                nc.tensor.matmul(ops[:, :], lhsT=wp[:, OFF_WO3 + o * P: OFF_WO3 + (o + 1) * P],
                                 rhs=h2[:, :], start=True, stop=True)
                ofin = spool.tile([P, BL], f32, tag=f"ofin{o}")
                nc.scalar.activation(ofin[:, :], ops[:, :], Act.Identity,
                                     bias=wp[:, OFF_B + 2 + o:OFF_B + 3 + o], scale=1.0)
                nc.sync.dma_start(out=outT[o * P:(o + 1) * P, :], in_=ofin[:, :])

    nc.compile()
    return nc


def _pack_weights(inputs):
    wv = np.asarray(inputs["wv"], np.float32)
    wo1 = np.asarray(inputs["wo1"], np.float32)
    wo2 = np.asarray(inputs["wo2"], np.float32)
    wo3 = np.asarray(inputs["wo3"], np.float32)
    b1 = np.asarray(inputs["b1"], np.float32)
    b2 = np.asarray(inputs["b2"], np.float32)
    b3 = np.asarray(inputs["b3"], np.float32)

    wvs = wv * np.float32(1.0 / NPTS)        # fold the mean scale into wv
    wpack = np.zeros((P, WPACK_F), np.float32)
    wpack[:, OFF_WVS:OFF_WVS + 1024] = wvs[0:128, :]
    wpack[:, OFF_WVS + 1024:OFF_WVS + 2048] = wvs[128:256, :]
    for k in range(8):
        wpack[:, OFF_WO1 + k * P:OFF_WO1 + (k + 1) * P] = wo1[k * P:(k + 1) * P, :]
    wpack[:, OFF_WO2:OFF_WO2 + P] = wo2
    wpack[:, OFF_WO3:OFF_WO3 + CH] = wo3
    wpack[:, OFF_B + 0] = b1
    wpack[:, OFF_B + 1] = b2
    wpack[:, OFF_B + 2] = b3[0:128]
    wpack[:, OFF_B + 3] = b3[128:256]
    wpack[:, OFF_ONE] = 1.0
    wpack[0, OFF_I2] = 1.0
    wpack[1, OFF_I2 + 1] = 1.0
    return wpack


def kernel(**inputs):
    from concourse.bass_utils import run_bass_kernel_spmd

    if "nc" not in _CACHE:
        _CACHE["nc"] = _build_program()
    nc = _CACHE["nc"]

    lidar = np.ascontiguousarray(np.asarray(inputs["lidar"], dtype=np.float32))
    wpack = _pack_weights(inputs)

    in_maps = [
        {"lidar": lidar[i * BL:(i + 1) * BL], "wpack": wpack}
        for i in range(N_CORES)
    ]
    res = run_bass_kernel_spmd(nc, in_maps, list(range(N_CORES)),
                               **_CACHE.get("run_kwargs", {}))
    _CACHE["last_results"] = res
    out = np.concatenate([res.results[i]["outT"].T for i in range(N_CORES)], axis=0)
    return np.ascontiguousarray(out, dtype=np.float32)
